# revision 1
# baseline (speedup 1.0000x reference)
"""Trainium2 Bass kernel for nn_Net_274877907721 (LSTM encoder + batched
decoder step + FC head).

Sharding: encoder 2-way data-parallel over batch (cores 0-3 take batch
0-31, cores 4-7 take batch 32-63; 4x replicated within each quad, with
each core's batch order permuted so its decoder slice is rows 0-7).
Decoder/FC 8-way data-parallel (8 batch rows per core).

Encoder recurrence: pre_t = [h | x_t | 1] @ [Whh.T ; Wih.T ; bias] as one
PSUM accumulation, 4-way column-tiled across PE col-groups (strip g =
gate g), bf16 operands / f32 accumulate+elementwise.
"""
import sys
import numpy as np

sys.path.insert(0, "/opt/trn_rl_repo")

import ml_dtypes
import concourse.bass as bass
import concourse.mybir as mybir
import concourse.tile as tile
from concourse import bacc
from concourse.bass_utils import run_bass_kernel_spmd

F32 = mybir.dt.float32
BF16 = mybir.dt.bfloat16
AF = mybir.ActivationFunctionType
ALU = mybir.AluOpType
BF = ml_dtypes.bfloat16

B, T, I, H, O = 64, 512, 256, 1024, 256
G4 = 4 * H
MB = 32          # encoder batch per core
DB = 8           # decoder batch per core
NCORES = 8

# strips: 0=i, 1=o, 2=f, 3=g  (torch gate blocks i,f,g,o = 0,1,2,3)
# strips i,o share psum windows {0,1}; f,g share {2,3} (phase-alternated)
STRIP2TORCH = [0, 3, 1, 2]

# encoder dynamic loop: peel t=0..7, loop t=8..503 (496 = 8x62), peel 504..511
PEEL_HEAD = 8
LOOP_START = 8
LOOP_END = int(__import__('os').environ.get('KERNEL_LOOP_END', '504'))
UNROLL = 8

_CACHED = {}

# (strip, chunk) -> psum window (free 512-block of the [128, 2048] ps tile)
def _win(s, c):
    return c if s < 2 else 2 + c

# phase -> list of (strip, chunk): all four windows distinct per phase
_PHASES = [[(0, 0), (1, 1), (2, 0), (3, 1)],
           [(0, 1), (1, 0), (2, 1), (3, 0)]]


def _gate_reorder():
    return np.concatenate([np.arange(s * H, (s + 1) * H) for s in STRIP2TORCH])


def _build():
    nc = bacc.Bacc(None, target_bir_lowering=False)

    # ---------------- I/O ----------------
    xT_enc = nc.dram_tensor("xT_enc", [T + 2, 128, 2, MB], BF16, kind="ExternalInput")
    whhT = nc.dram_tensor("whhT", [128, 8, G4], BF16, kind="ExternalInput")
    wihT = nc.dram_tensor("wihT", [128, 2, G4], BF16, kind="ExternalInput")
    biasW = nc.dram_tensor("biasW", [128, G4], BF16, kind="ExternalInput")   # row0 = enc bias (reordered)
    onesW = nc.dram_tensor("onesW", [128, 128], BF16, kind="ExternalInput")  # row0 = ones
    ident = nc.dram_tensor("ident", [32, 32], F32, kind="ExternalInput")

    dwihT = nc.dram_tensor("dwihT", [128, 2, G4], BF16, kind="ExternalInput")
    dwhhT = nc.dram_tensor("dwhhT", [128, 8, G4], BF16, kind="ExternalInput")
    dbias = nc.dram_tensor("dbias", [128, G4], BF16, kind="ExternalInput")
    xT_dec = nc.dram_tensor("xT_dec", [2, 128, DB, T], BF16, kind="ExternalInput")
    indPad = nc.dram_tensor("indPad", [128, DB, T], BF16, kind="ExternalInput")  # rows0-7 indicator
    fcWT = nc.dram_tensor("fcWT", [128, 8, O], BF16, kind="ExternalInput")
    fcbW = nc.dram_tensor("fcbW", [128, O], BF16, kind="ExternalInput")      # row0 = fc bias

    pred = nc.dram_tensor("pred", [DB, T, O], F32, kind="ExternalOutput")

    with tile.TileContext(nc) as tc:
        with (
            tc.tile_pool(name="dram", bufs=1, space="DRAM") as dram,
            tc.tile_pool(name="state", bufs=1) as state,
        ):
            hnT_dram = dram.tile([8, 128, DB, T], BF16)

            # long-lived state (survives into decoder)
            tgc = state.tile([64, H], F32)        # rows0-31 tanh(g), rows32-63 c
            idn = state.tile([32, 32], F32)
            nc.sync.dma_start(idn[:, :], ident[:, :])
            hT_hold = state.tile([128, 8, MB], BF16)  # final-step hT for decoder
            cT = state.tile([128, 8, DB], F32)

            # ============= ENCODER =============
            with (
                tc.tile_pool(name="encconst", bufs=1) as encconst,
                tc.tile_pool(name="encpsum", bufs=1, space="PSUM") as psum,
            ):
                whhT_sb = encconst.tile([128, 8, G4], BF16)
                wihT_sb = encconst.tile([128, 2, G4], BF16)
                biasW_sb = encconst.tile([128, G4], BF16)
                onesW_sb = encconst.tile([128, 128], BF16)
                nc.sync.dma_start(whhT_sb[:, :, :], whhT[:, :, :])
                nc.sync.dma_start(wihT_sb[:, :, :], wihT[:, :, :])
                nc.sync.dma_start(biasW_sb[:, :], biasW[:, :])
                nc.sync.dma_start(onesW_sb[:, :], onesW[:, :])

                sif = encconst.tile([64, H], F32)    # sig(i)@p0, sig(o)@p32
                sfa = encconst.tile([64, H], F32)    # rows32-63: sig(f)@p32
                hp = encconst.tile([64, H], F32)     # rows32-63: tanh(c)@p32
                h_sb = encconst.tile([32, H], F32)
                prods = encconst.tile([64, H], F32)  # rows32-63: i*g @p32
                prods2 = encconst.tile([64, H], F32)  # rows32-63: f*c @p32

                # explicit rings (slot = t mod ring; trace-static because
                # LOOP_START % ring == 0 and UNROLL % ring == 0)
                xt_ring = [encconst.tile([128, 2, MB], BF16, name=f"xtr{i}")
                           for i in range(4)]
                hT_ring = [encconst.tile([128, 8, MB], BF16, name=f"hTr{i}")
                           for i in range(2)]
                ps_ring = [psum.tile([128, 2048], F32, name=f"psr{i}")
                           for i in range(2)]

                def load_xt(idx_expr, slot):
                    nc.sync.dma_start(
                        xt_ring[slot][:, :, :],
                        xT_enc[idx_expr, :, :, :],
                    )

                def emit_k(ps, lhsT, rhsW, kslice, start, stop):
                    # one contraction k-tile: 2 phases x 4 strips, N=512 each,
                    # all four psum windows distinct within a phase
                    for phase in _PHASES:
                        for (st, ch) in phase:
                            nc.tensor.matmul(
                                ps[32 * st:32 * st + 32,
                                   bass.ts(_win(st, ch), 512)],
                                lhsT,
                                rhsW[:, kslice, bass.ds(st * H + ch * 512, 512)],
                                start=start, stop=stop,
                                tile_position=(0, 32 * st))

                def mm_step(first_step, xt, hT_prev, ps):
                    emit_k(ps, xt[:, 0, :], wihT_sb, 0, True, False)
                    emit_k(ps, xt[:, 1, :], wihT_sb, 1, False, False)
                    emit_k(ps, onesW_sb[:, 0:MB], biasW_sb[:, None, :], 0,
                           False, first_step)
                    if not first_step:
                        for k in range(8):
                            emit_k(ps, hT_prev[:, k, :], whhT_sb, k,
                                   False, k == 7)

                def chain(first_step, ps, slot2, keep_hT=False):
                    # gates: i=ps[0:32, 0:1024], o=ps[32:64, 0:1024],
                    #        f=ps[64:96, 1024:2048], g=ps[96:128, 1024:2048]
                    nc.scalar.activation(sif[:, :], ps[0:64, 0:1024], AF.Sigmoid)
                    nc.scalar.activation(sfa[32:64, :], ps[64:96, 1024:2048],
                                         AF.Sigmoid)
                    nc.scalar.activation(tgc[0:32, :], ps[96:128, 1024:2048],
                                         AF.Tanh)
                    if first_step:
                        # c = i*g  (cross-base out p0 -> p32)
                        nc.vector.tensor_tensor(tgc[32:64, :], sif[0:32, :],
                                                tgc[0:32, :], op=ALU.mult)
                    else:
                        nc.vector.tensor_tensor(prods[32:64, :], sif[0:32, :],
                                                tgc[0:32, :], op=ALU.mult)
                        nc.vector.tensor_tensor(prods2[32:64, :], sfa[32:64, :],
                                                tgc[32:64, :], op=ALU.mult)
                        nc.vector.tensor_tensor(tgc[32:64, :], prods[32:64, :],
                                                prods2[32:64, :], op=ALU.add)
                    nc.scalar.activation(hp[32:64, :], tgc[32:64, :], AF.Tanh)
                    nc.vector.tensor_tensor(h_sb[:, :], sif[32:64, :],
                                            hp[32:64, :], op=ALU.mult)
                    # transposes write into spare psum cells (window0 first 1KB)
                    tp = ps[:, 0:256].rearrange("p (k m) -> p k m", k=8)
                    for k in range(8):
                        nc.tensor.transpose(tp[:, k, :], h_sb[:, bass.ts(k, 128)],
                                            idn[:, :])
                    hT = hT_hold if keep_hT else hT_ring[slot2]
                    nc.vector.tensor_copy(hT[:, :, :], tp[:, :, :])

                # ---- peeled head t = 0..7 ----
                load_xt(0, 0)
                load_xt(1, 1)
                for t in range(PEEL_HEAD):
                    load_xt(t + 2, (t + 2) % 4)
                    ps = ps_ring[t % 2]
                    mm_step(t == 0, xt_ring[t % 4],
                            hT_ring[(t - 1) % 2] if t else None, ps)
                    chain(t == 0, ps, t % 2)

                # ---- dynamic loop t = 8..503 ----
                def body(iv, j=[0]):
                    t = j[0] % UNROLL  # trace-static phase (iv = 8 + 8*pass)
                    j[0] += 1
                    load_xt(iv + 2, (t + 2) % 4)
                    ps = ps_ring[t % 2]
                    mm_step(False, xt_ring[t % 4], hT_ring[(t - 1) % 2], ps)
                    chain(False, ps, t % 2)

                if LOOP_END > LOOP_START:
                    tc.For_i_unrolled(LOOP_START, LOOP_END, 1, body,
                                      max_unroll=UNROLL)

                # ---- peeled tail t = 504..511 ----
                for t in range(LOOP_END, T):
                    load_xt(t + 2, (t + 2) % 4)
                    ps = ps_ring[t % 2]
                    mm_step(False, xt_ring[t % 4], hT_ring[(t - 1) % 2], ps)
                    chain(False, ps, t % 2, keep_hT=(t == T - 1))

                # c -> cT tiles [128, 8, DB] f32 for decoder
                # (copy c to a base-0 tile first: transpose needs base match)
                nc.vector.tensor_copy(h_sb[:, :], tgc[32:64, :])
                tpc = ps_ring[0][:, 0:256].rearrange("p (k m) -> p k m", k=8)
                for k in range(8):
                    nc.tensor.transpose(tpc[:, k, :], h_sb[:, bass.ts(k, 128)],
                                        idn[:, :])
                nc.vector.tensor_copy(cT[:, :, :], tpc[:, :, 0:DB])

            # ============= DECODER =============
            with (
                tc.tile_pool(name="decconst", bufs=1) as decconst,
                tc.tile_pool(name="decwork", bufs=2) as dwork,
            ):
                dwihT_sb = decconst.tile([128, 2, G4], BF16)
                dwhhT_sb = decconst.tile([128, 8, G4], BF16)
                dbiasW_sb = decconst.tile([128, G4], BF16)
                xTd_sb = decconst.tile([128, 2, DB, T], BF16)
                ind_sb = decconst.tile([128, DB, T], BF16)
                onesD_sb = decconst.tile([128, 128], BF16)
                nc.sync.dma_start(dwihT_sb[:, :, :], dwihT[:, :, :])
                nc.sync.dma_start(dwhhT_sb[:, :, :], dwhhT[:, :, :])
                nc.sync.dma_start(dbiasW_sb[:, :], dbias[:, :])
                nc.sync.dma_start(xTd_sb[:, 0, :, :], xT_dec[0, :, :, :])
                nc.sync.dma_start(xTd_sb[:, 1, :, :], xT_dec[1, :, :, :])
                nc.sync.dma_start(ind_sb[:, :, :], indPad[:, :, :])
                nc.sync.dma_start(onesD_sb[:, :], onesW[:, :])

                # hpre[b, :] = h_dec @ dec_Whh.T + dec_bias  -> [128, G4] rows0-7
                hpre_sb = decconst.tile([128, G4], BF16)
                nc.scalar.memzero(hpre_sb[:, :])
                with tc.tile_pool(name="psA", bufs=1, space="PSUM") as psA:
                    for half in range(8):
                        psh = psA.tile([DB, 512], F32, tag="psh", bufs=2)
                        for k in range(8):
                            nc.tensor.matmul(
                                psh[:, :],
                                hT_hold[:, k, 0:DB],
                                dwhhT_sb[:, k, bass.ts(half, 512)],
                                start=(k == 0), stop=False,
                                skip_group_check=True,
                            )
                        # += bias via ones-row matmul (padded to K=128)
                        nc.tensor.matmul(psh[:, :],
                                         onesD_sb[:, 0:DB],
                                         dbiasW_sb[:, bass.ts(half, 512)],
                                         start=False, stop=True,
                                         skip_group_check=True)
                        nc.scalar.copy(hpre_sb[0:DB, bass.ts(half, 512)], psh[:, :])

                # main gate loop: hq = h-dim quad (128 cols), bp = batch pair
                with tc.tile_pool(name="psB", bufs=1, space="PSUM") as psB:
                  for hq in range(8):
                    cbc = cT[:, hq, :]
                    for bp in range(4):
                        pd_if = psB.tile([128, 2048], F32, tag="pdif", bufs=1)
                        pd_og = psB.tile([128, 2048], F32, tag="pdog", bufs=1)
                        for kk in range(3):  # contraction: x k0, x k1, hpre
                            for jn in range(2):
                                for gi in range(4):
                                    pd = pd_if if gi < 2 else pd_og
                                    torch_g = (0, 1, 3, 2)[gi]  # i, f, o, g
                                    colbase = torch_g * H + hq * 128
                                    half = gi % 2
                                    dst = pd[:, bass.ds(half * 1024 + jn * 512, 512)]
                                    rsl = bass.ds(bp * 2 * T + jn * 512, 512)
                                    if kk < 2:
                                        lhsT = dwihT_sb[:, kk, bass.ds(colbase, 128)]
                                        rhs = xTd_sb[:, kk, :, :].rearrange("p b t -> p (b t)")[:, rsl]
                                    else:
                                        lhsT = hpre_sb[:, bass.ds(colbase, 128)]
                                        rhs = ind_sb.rearrange("p b t -> p (b t)")[:, rsl]
                                    nc.tensor.matmul(
                                        dst, lhsT, rhs,
                                        start=(kk == 0), stop=(kk == 2),
                                        skip_group_check=True)
                        sif_d = dwork.tile([128, 2048], F32, tag="sifd")
                        nc.scalar.activation(sif_d[:, :], pd_if[:, :], AF.Sigmoid)
                        so_d = dwork.tile([128, 1024], F32, tag="sod")
                        nc.scalar.activation(so_d[:, :], pd_og[:, 0:1024], AF.Sigmoid)
                        tg_d = dwork.tile([128, 1024], F32, tag="tgd")
                        nc.scalar.activation(tg_d[:, :], pd_og[:, 1024:2048], AF.Tanh)
                        ig_d = dwork.tile([128, 1024], F32, tag="igd")
                        nc.vector.tensor_tensor(ig_d[:, :], sif_d[:, 0:1024],
                                                tg_d[:, :], op=ALU.mult)
                        fc_d = dwork.tile([128, 1024], F32, tag="fcd")
                        nc.vector.tensor_tensor(
                            fc_d.rearrange("p (b t) -> p b t", b=2),
                            sif_d[:, 1024:2048].rearrange("p (b t) -> p b t", b=2),
                            cbc[:, bass.ds(bp * 2, 2), None].broadcast_to([128, 2, T]),
                            op=ALU.mult)
                        cn_d = dwork.tile([128, 1024], F32, tag="cnd")
                        nc.vector.tensor_tensor(cn_d[:, :], ig_d[:, :], fc_d[:, :],
                                                op=ALU.add)
                        tc_d = dwork.tile([128, 1024], F32, tag="tcd")
                        nc.scalar.activation(tc_d[:, :], cn_d[:, :], AF.Tanh)
                        hn_d = dwork.tile([128, 1024], BF16, tag="hnd")
                        nc.vector.tensor_tensor(hn_d[:, :], so_d[:, :], tc_d[:, :],
                                                op=ALU.mult)
                        nc.sync.dma_start(
                            hnT_dram[hq, :, bass.ds(bp * 2, 2), :],
                            hn_d.rearrange("p (b t) -> p b t", b=2))

                # fc: pred[rows, O] = hnT.T @ fcW.T + fc_b
                fcWT_sb = decconst.tile([128, 8, O], BF16)
                fcb_sb = decconst.tile([128, O], BF16)
                nc.sync.dma_start(fcWT_sb[:, :, :], fcWT[:, :, :])
                nc.sync.dma_start(fcb_sb[:, :], fcbW[:, :])
                with tc.tile_pool(name="psC", bufs=1, space="PSUM") as psC:
                  for b in range(DB):
                    for tb in range(4):
                        fcin = dwork.tile([128, 8, 128], BF16, tag="fcin", bufs=3)
                        nc.sync.dma_start(
                            fcin[:, :, :],
                            hnT_dram[:, :, b, bass.ts(tb, 128)].rearrange("k p t -> p k t"))
                        pf = psC.tile([128, O], F32, tag="pf", bufs=2)
                        for k in range(8):
                            nc.tensor.matmul(pf[:, :], fcin[:, k, :],
                                             fcWT_sb[:, k, :],
                                             start=(k == 0), stop=False,
                                             skip_group_check=True)
                        nc.tensor.matmul(pf[:, :], onesD_sb[:, 0:128],
                                         fcb_sb[:, :],
                                         start=False, stop=True,
                                         skip_group_check=True)
                        out_sb = dwork.tile([128, O], F32, tag="outsb", bufs=3)
                        nc.scalar.copy(out_sb[:, :], pf[:, :])
                        nc.sync.dma_start(
                            pred[b, bass.ts(tb, 128), :], out_sb[:, :])

    nc.compile()
    return nc


def _prep_core_inputs(core, x, enc_Wih, enc_Whh, enc_bih, enc_bhh,
                      dec_Wih, dec_Whh, dec_bih, dec_bhh, fc_W, fc_b):
    half = core // 4
    off = (8 * core) % 32
    perm = np.concatenate([np.arange(off, off + 8),
                           np.array([j for j in range(32)
                                     if not (off <= j < off + 8)], dtype=int)])
    xc = x[half * 32:(half + 1) * 32][perm]          # [32, T, I]

    R = _gate_reorder()

    xT = np.ascontiguousarray(np.transpose(xc, (1, 2, 0)))   # [T, I, 32]
    xT_enc = np.zeros((T + 2, 128, 2, MB), dtype=BF)
    xT_enc[:T] = np.transpose(xT.reshape(T, 2, 128, MB), (0, 2, 1, 3)).astype(BF)

    def ktiles(wT, nk):
        # wT: [K, N] -> [128, nk, N]
        return np.ascontiguousarray(
            np.transpose(wT.reshape(nk, 128, wT.shape[1]), (1, 0, 2))).astype(BF)

    whhT = ktiles(enc_Whh[R].T, 8)           # [128, 8, 4096]
    wihT = ktiles(enc_Wih[R].T, 2)
    biasW = np.zeros((128, G4), dtype=BF)
    biasW[0] = (enc_bih + enc_bhh)[R].astype(BF)
    onesW = np.zeros((128, 128), dtype=BF)
    onesW[0] = 1.0
    ident = np.eye(32, dtype=np.float32)

    dwihT = ktiles(dec_Wih.T, 2)
    dwhhT = ktiles(dec_Whh.T, 8)
    dbias = np.zeros((128, G4), dtype=BF)
    dbias[0] = (dec_bih + dec_bhh).astype(BF)
    xT_dec = np.ascontiguousarray(
        np.transpose(xc[:8], (2, 0, 1))).reshape(2, 128, DB, T).astype(BF)
    indPad = np.zeros((128, DB, T), dtype=BF)
    for b in range(DB):
        indPad[b, b, :] = 1.0
    fcWT = ktiles(fc_W.T, 8)                 # [128, 8, 256]
    fcbW = np.zeros((128, O), dtype=BF)
    fcbW[0] = fc_b.astype(BF)

    return {
        "xT_enc": xT_enc, "whhT": whhT, "wihT": wihT, "biasW": biasW,
        "onesW": onesW, "ident": ident,
        "dwihT": dwihT, "dwhhT": dwhhT, "dbias": dbias,
        "xT_dec": xT_dec, "indPad": indPad, "fcWT": fcWT, "fcbW": fcbW,
    }


def kernel(**inputs):
    args = {k: np.asarray(v) for k, v in inputs.items()}
    if "nc" not in _CACHED:
        _CACHED["nc"] = _build()
    nc = _CACHED["nc"]
    in_maps = [_prep_core_inputs(c, **args) for c in range(NCORES)]
    res = run_bass_kernel_spmd(nc, in_maps, core_ids=list(range(NCORES)))
    preds = [res.results[c]["pred"] for c in range(NCORES)]
    return np.concatenate(preds, axis=0)        # [64, T, O]


if __name__ == "__main__":
    rng = np.random.default_rng(0)
    ins = {
        "x": rng.standard_normal((B, T, I), dtype=np.float32),
        "enc_Wih": rng.standard_normal((G4, I), dtype=np.float32) * 0.03,
        "enc_Whh": rng.standard_normal((G4, H), dtype=np.float32) * 0.03,
        "enc_bih": rng.standard_normal(G4).astype(np.float32) * 0.03,
        "enc_bhh": rng.standard_normal(G4).astype(np.float32) * 0.03,
        "dec_Wih": rng.standard_normal((G4, I), dtype=np.float32) * 0.03,
        "dec_Whh": rng.standard_normal((G4, H), dtype=np.float32) * 0.03,
        "dec_bih": rng.standard_normal(G4).astype(np.float32) * 0.03,
        "dec_bhh": rng.standard_normal(G4).astype(np.float32) * 0.03,
        "fc_W": rng.standard_normal((O, H), dtype=np.float32) * 0.03,
        "fc_b": rng.standard_normal(O).astype(np.float32) * 0.03,
    }
    out = kernel(**ins)
    print(out.shape, out.dtype, np.abs(out).mean())

